# revision 2
# baseline (speedup 1.0000x reference)
"""Bahdanau-style additive attention on 8 TRN2 NeuronCores.

  hidden = tanh(q @ Wq + k @ Wk)        (B, L, H)
  scores = hidden @ v_param             (B, L)
  attn   = softmax(scores, axis=-1)
  out    = attn @ v                     (B, D)

Sharding: data-parallel over batch — 4 batches per core (B=32, 8 cores).

Per-core device pipeline (all heavy matmuls in float32r, the PE's
TF32-like 4-byte mode: ~11-bit-mantissa RNE inputs, fp32 accumulate):

  W1  preT[H, L]   = Wk.T @ kT          stationary=Wk, moving=host-transposed k
  ACT hiddenT      = tanh(preT + qWq_b) per-partition bias, f32r output
  W2  scores[L, 1] = hiddenT.T @ vp     stationary=hidden chunk -> score COLUMNS
  ACT w = exp(scores)                   no max-subtraction (|scores| << 88)
  W3  acc[1, D+1]  = w.T @ [v | 1]      stationary=w column, ones column gives
                                        the softmax denominator for free
  host: out = acc[:D] / acc[D]
"""

import numpy as np

import concourse.bass as bass
import concourse.mybir as mybir
from concourse.tile import TileContext

B, L, D, H = 32, 8192, 128, 128
NCORES = 8
BPC = B // NCORES  # batches per core
CHUNK = 512  # L positions per W1/tanh chunk
NCH = L // CHUNK  # 16 chunks per batch
SUB = 128  # L positions per W2/W3 sub-chunk (stationary width)
NSUB = CHUNK // SUB  # 4
DV = 132  # v row: 128 data + ones col + 3 pad (16B-aligned rows)
VT_COLS = 16  # W3 sub-chunks per v SBUF tile
NVT = L // (SUB * VT_COLS)  # 4 v tiles per batch

F32 = mybir.dt.float32
F32R = mybir.dt.float32r
ACTF = mybir.ActivationFunctionType

_CACHE = {}


def _split_excess_waits(nc, max_waits=1):
    """walrus in this env accepts at most one sync-wait per instruction;
    move extras onto InstNoOps placed just before (same engine, in order)."""
    for fn in nc.m.functions:
        for bb in fn.blocks:
            insts = list(bb.instructions)
            new_insts = []
            for ins in insts:
                si = ins.sync_info
                waits = list(si.on_wait) if si and si.on_wait else []
                if len(waits) > max_waits:
                    extra, keep = waits[:-max_waits], waits[-max_waits:]
                    for g0 in range(0, len(extra), max_waits):
                        pre = mybir.InstNoOp(
                            name=f"{ins.name}-waitsplit{g0}",
                            engine=ins.engine,
                            ins=[],
                            outs=[],
                            sync_info=mybir.SyncInfo(
                                on_wait=extra[g0 : g0 + max_waits], on_update=[]
                            ),
                        )
                        nc.register_instruction(pre, overwrite=True)
                        new_insts.append(pre)
                    ins.sync_info = mybir.SyncInfo(
                        on_wait=keep, on_update=list(si.on_update or [])
                    )
                new_insts.append(ins)
            if len(new_insts) != len(insts):
                bb.instructions[:] = new_insts


def build_nc():
    nc = bass.Bass("TRN2")

    kT_in = nc.dram_tensor("kT", [BPC, D, L], F32R, kind="ExternalInput")
    v_in = nc.dram_tensor("vv", [BPC, L, DV], F32R, kind="ExternalInput")
    wk_in = nc.dram_tensor("wk", [D, H], F32R, kind="ExternalInput")
    vp_in = nc.dram_tensor("vp", [H, 4], F32R, kind="ExternalInput")
    qwq_in = nc.dram_tensor("qwq", [H, BPC], F32, kind="ExternalInput")
    out_d = nc.dram_tensor("out", [1, BPC * DV], F32, kind="ExternalOutput")

    with TileContext(nc) as tc:
        with (
            tc.tile_pool(name="const", bufs=1) as cpool,
            tc.tile_pool(name="kp", bufs=4) as kpool,
            tc.tile_pool(name="vp_", bufs=2 * NVT) as vpool,
            tc.tile_pool(name="hp", bufs=3) as hpool,
            tc.tile_pool(name="sc", bufs=2) as scpool,
            tc.tile_pool(name="wp", bufs=2) as wpool,
            tc.tile_pool(name="ob", bufs=1) as opool,
            tc.tile_pool(name="pre", bufs=2, space="PSUM") as pre_pool,
            tc.tile_pool(name="sps", bufs=2, space="PSUM") as s_pool,
            tc.tile_pool(name="ops", bufs=2, space="PSUM") as o_pool,
        ):
            wk = cpool.tile([D, H], F32R)
            vp4 = cpool.tile([H, 4], F32R)
            qwq = cpool.tile([H, BPC], F32)
            nc.sync.dma_start(wk[:], wk_in[:])
            nc.sync.dma_start(vp4[:], vp_in[:])
            nc.sync.dma_start(qwq[:], qwq_in[:])

            out_sb = opool.tile([1, BPC * DV], F32)

            for b in range(BPC):
                # v~ tiles for this batch: [128, VT_COLS*DV] each
                v_tiles = []
                for vt in range(NVT):
                    vtile = vpool.tile([SUB, VT_COLS * DV], F32R, tag="vt")
                    src = v_in[b, vt * SUB * VT_COLS : (vt + 1) * SUB * VT_COLS, :]
                    nc.sync.dma_start(
                        vtile[:].rearrange("p (t d) -> p t d", d=DV),
                        src.rearrange("(t p) d -> p t d", p=SUB),
                    )
                    v_tiles.append(vtile)

                scores = scpool.tile([SUB, L // SUB], F32, tag="scores")
                for t in range(NCH):
                    kt = kpool.tile([D, CHUNK], F32R, tag="kt")
                    nc.sync.dma_start(
                        kt[:], kT_in[b, :, t * CHUNK : (t + 1) * CHUNK]
                    )
                    pre = pre_pool.tile([H, CHUNK], F32, tag="pre")
                    nc.tensor.matmul(pre[:], wk[:], kt[:], start=True, stop=True)
                    hid = hpool.tile([H, CHUNK], F32R, tag="hid")
                    nc.scalar.activation(
                        hid[:], pre[:], ACTF.Tanh, bias=qwq[:, b : b + 1], scale=1.0
                    )
                    scol = s_pool.tile([SUB, 4 * NSUB], F32, tag="scol")
                    for j in range(NSUB):
                        nc.tensor.matmul(
                            scol[:, 4 * j : 4 * j + 4],
                            hid[:, j * SUB : (j + 1) * SUB],
                            vp4[:],
                            start=True,
                            stop=True,
                        )
                    # gather the 4 useful columns {0,4,8,12} -> scores[:, 4t..4t+4)
                    nc.vector.tensor_copy(
                        scores[:, 4 * t : 4 * t + 4], scol[:, 0 : 4 * NSUB : 4]
                    )

                w = wpool.tile([SUB, L // SUB], F32R, tag="w")
                nc.scalar.activation(w[:], scores[:], ACTF.Exp)

                acc = o_pool.tile([1, DV], F32, tag="acc")
                nsub_total = L // SUB
                for tp in range(nsub_total):
                    vt, col = divmod(tp, VT_COLS)
                    nc.tensor.matmul(
                        acc[:],
                        w[:, tp : tp + 1],
                        v_tiles[vt][:, col * DV : (col + 1) * DV],
                        start=(tp == 0),
                        stop=(tp == nsub_total - 1),
                    )
                nc.scalar.copy(out_sb[:, b * DV : (b + 1) * DV], acc[:])

            nc.sync.dma_start(out_d[:], out_sb[:])

    _split_excess_waits(nc)
    return nc


def _prep_inputs(q, k, v, W_line, v_param):
    """Host-side shard + layout prep. Returns per-core input maps."""
    qWq = q.astype(np.float64) @ W_line[:D].astype(np.float64)  # (B, H)
    wk = np.ascontiguousarray(W_line[D:])  # (D, H)
    vp4 = np.tile(v_param[:, None], (1, 4))  # (H, 4)

    in_maps = []
    for c in range(NCORES):
        bs = slice(c * BPC, (c + 1) * BPC)
        kT = np.ascontiguousarray(k[bs].transpose(0, 2, 1))  # (BPC, D, L)
        vv = np.zeros((BPC, L, DV), dtype=np.float32)
        vv[:, :, :D] = v[bs]
        vv[:, :, D] = 1.0
        qwq = np.ascontiguousarray(qWq[bs].T.astype(np.float32))  # (H, BPC)
        in_maps.append(
            {
                "kT": kT,
                "vv": vv,
                "wk": wk.astype(np.float32),
                "vp": vp4.astype(np.float32),
                "qwq": qwq,
            }
        )
    return in_maps


def _gather_output(results):
    out = np.empty((B, D), dtype=np.float32)
    for c, r in enumerate(results):
        rows = r["out"].reshape(BPC, DV).astype(np.float64)
        out[c * BPC : (c + 1) * BPC] = (rows[:, :D] / rows[:, D : D + 1]).astype(
            np.float32
        )
    return out


def run(q, k, v, W_line, v_param, trace=False, **spmd_kwargs):
    from concourse.bass_utils import run_bass_kernel_spmd

    if "nc" not in _CACHE:
        _CACHE["nc"] = build_nc()
    nc = _CACHE["nc"]
    in_maps = _prep_inputs(q, k, v, W_line, v_param)
    res = run_bass_kernel_spmd(
        nc, in_maps, list(range(NCORES)), trace=trace, **spmd_kwargs
    )
    return _gather_output(res.results), res


def kernel(q, k, v, W_line, v_param):
    out, _ = run(q, k, v, W_line, v_param, trace=False)
    return out


# revision 6
# speedup vs baseline: 1.2630x; 1.2630x over previous
"""Bahdanau-style additive attention on 8 TRN2 NeuronCores.

  hidden = tanh(q @ Wq + k @ Wk)        (B, L, H)
  scores = hidden @ v_param             (B, L)
  attn   = softmax(scores, axis=-1)
  out    = attn @ v                     (B, D)

Sharding: data-parallel over batch — 4 batches per core (B=32, 8 cores).

Per-core device pipeline (all heavy matmuls in float32r, the PE's
TF32-like 4-byte mode: ~11-bit-mantissa RNE inputs, fp32 accumulate):

  W1  preT[H, L]   = Wk.T @ kT          stationary=Wk, moving=host-transposed k
  ACT hiddenT      = tanh(preT + qWq_b) per-partition bias, f32r output
  W2  scores[L, 1] = hiddenT.T @ vp     stationary=hidden chunk -> score COLUMNS
  ACT w = exp(scores)                   no max-subtraction (|scores| << 88)
  W3  acc[1, D+1]  = w.T @ [v | 1]      stationary=w column, ones column gives
                                        the softmax denominator for free
  host: out = acc[:D] / acc[D]
"""

import numpy as np

import concourse.bass as bass
import concourse.mybir as mybir
from concourse.tile import TileContext

B, L, D, H = 32, 8192, 128, 128
NCORES = 8
BPC = B // NCORES  # batches per core
CHUNK = 512  # L positions per W1/tanh chunk (f32r moving max)
NCH = L // CHUNK  # 16 chunks per batch
KTILE = 2048  # L positions per kT DMA tile (1 MB transfers)
KCH = KTILE // CHUNK  # W1 chunks per kT tile
SUB = 128  # L positions per W2/W3 sub-chunk (stationary width)
NSUB = CHUNK // SUB  # 4
DV = 132  # v row: 128 data + ones col + 3 pad (16B-aligned rows)
VT_COLS = 16  # W3 sub-chunks per v SBUF tile
NVT = L // (SUB * VT_COLS)  # 4 v tiles per batch

F32 = mybir.dt.float32
F32R = mybir.dt.float32r
ACTF = mybir.ActivationFunctionType

_CACHE = {}


def _split_excess_waits(nc, max_waits=1):
    """walrus in this env accepts at most one sync-wait per instruction;
    move extras onto InstNoOps placed just before (same engine, in order)."""
    for fn in nc.m.functions:
        for bb in fn.blocks:
            insts = list(bb.instructions)
            new_insts = []
            for ins in insts:
                si = ins.sync_info
                waits = list(si.on_wait) if si and si.on_wait else []
                if len(waits) > max_waits:
                    extra, keep = waits[:-max_waits], waits[-max_waits:]
                    for g0 in range(0, len(extra), max_waits):
                        pre = mybir.InstNoOp(
                            name=f"{ins.name}-waitsplit{g0}",
                            engine=ins.engine,
                            ins=[],
                            outs=[],
                            sync_info=mybir.SyncInfo(
                                on_wait=extra[g0 : g0 + max_waits], on_update=[]
                            ),
                        )
                        nc.register_instruction(pre, overwrite=True)
                        new_insts.append(pre)
                    ins.sync_info = mybir.SyncInfo(
                        on_wait=keep, on_update=list(si.on_update or [])
                    )
                new_insts.append(ins)
            if len(new_insts) != len(insts):
                bb.instructions[:] = new_insts


def build_nc():
    nc = bass.Bass("TRN2")

    kT_in = nc.dram_tensor("kT", [BPC, D, L], F32R, kind="ExternalInput")
    v_in = nc.dram_tensor("vv", [BPC, L, DV], F32R, kind="ExternalInput")
    wk_in = nc.dram_tensor("wk", [D, H], F32R, kind="ExternalInput")
    vp_in = nc.dram_tensor("vp", [H, 4], F32R, kind="ExternalInput")
    qwq_in = nc.dram_tensor("qwq", [H, BPC], F32, kind="ExternalInput")
    out_d = nc.dram_tensor("out", [1, BPC * DV], F32, kind="ExternalOutput")

    with TileContext(nc) as tc:
        with (
            tc.tile_pool(name="const", bufs=1) as cpool,
            tc.tile_pool(name="kp", bufs=4) as kpool,
            tc.tile_pool(name="vp_", bufs=2 * NVT) as vpool,
            tc.tile_pool(name="hp", bufs=3) as hpool,
            tc.tile_pool(name="sc", bufs=2) as scpool,
            tc.tile_pool(name="wp", bufs=2) as wpool,
            tc.tile_pool(name="ob", bufs=1) as opool,
            tc.tile_pool(name="pre", bufs=2, space="PSUM") as pre_pool,
            tc.tile_pool(name="sps", bufs=2, space="PSUM") as s_pool,
            tc.tile_pool(name="ops", bufs=2, space="PSUM") as o_pool,
        ):
            wk = cpool.tile([D, H], F32R)
            vp4 = cpool.tile([H, 4], F32R)
            qwq = cpool.tile([H, BPC], F32)
            nc.sync.dma_start(wk[:], wk_in[:])
            nc.sync.dma_start(vp4[:], vp_in[:])
            nc.sync.dma_start(qwq[:], qwq_in[:])

            out_sb = opool.tile([1, BPC * DV], F32)

            for b in range(BPC):
                # v~ tiles for this batch (SWDGE queue so the large v
                # prefetches never head-of-line-block the kT stream, whose
                # issue rate is throttled by PE back-pressure)
                v_tiles = [None] * NVT

                scores = scpool.tile([SUB, L // SUB], F32, tag="scores")
                kts = {}
                for t in range(NCH):
                    if t % KCH == 0:
                        ktile = kpool.tile([D, KTILE], F32R, tag="kt")
                        nc.sync.dma_start(
                            ktile[:], kT_in[b, :, t * CHUNK : t * CHUNK + KTILE]
                        )
                        kts[t // KCH] = ktile
                    if t % 4 == 1:
                        vt = t // 4
                        vtile = vpool.tile([SUB, VT_COLS * DV], F32R, tag="vt")
                        src = v_in[
                            b, vt * SUB * VT_COLS : (vt + 1) * SUB * VT_COLS, :
                        ]
                        nc.gpsimd.dma_start(
                            vtile[:].rearrange("p (t d) -> p t d", d=DV),
                            src.rearrange("(t p) d -> p t d", p=SUB),
                        )
                        v_tiles[vt] = vtile
                    kt = kts[t // KCH][:, (t % KCH) * CHUNK : (t % KCH + 1) * CHUNK]
                    pre = pre_pool.tile([H, CHUNK], F32, tag="pre")
                    nc.tensor.matmul(pre[:], wk[:], kt, start=True, stop=True)
                    hid = hpool.tile([H, CHUNK], F32R, tag="hid")
                    nc.scalar.activation(
                        hid[:], pre[:], ACTF.Tanh, bias=qwq[:, b : b + 1], scale=1.0
                    )
                    scol = s_pool.tile([SUB, 4 * NSUB], F32, tag="scol")
                    for j in range(NSUB):
                        nc.tensor.matmul(
                            scol[:, 4 * j : 4 * j + 4],
                            hid[:, j * SUB : (j + 1) * SUB],
                            vp4[:],
                            start=True,
                            stop=True,
                        )
                    # gather the 4 useful columns {0,4,8,12} -> scores[:, 4t..4t+4)
                    nc.vector.tensor_copy(
                        scores[:, 4 * t : 4 * t + 4], scol[:, 0 : 4 * NSUB : 4]
                    )

                w = wpool.tile([SUB, L // SUB], F32R, tag="w")
                nc.scalar.activation(w[:], scores[:], ACTF.Exp)

                acc = o_pool.tile([1, DV], F32, tag="acc")
                nsub_total = L // SUB
                for tp in range(nsub_total):
                    vt, col = divmod(tp, VT_COLS)
                    nc.tensor.matmul(
                        acc[:],
                        w[:, tp : tp + 1],
                        v_tiles[vt][:, col * DV : (col + 1) * DV],
                        start=(tp == 0),
                        stop=(tp == nsub_total - 1),
                    )
                nc.scalar.copy(out_sb[:, b * DV : (b + 1) * DV], acc[:])

            nc.sync.dma_start(out_d[:], out_sb[:])

    _split_excess_waits(nc)
    return nc


def _prep_inputs(q, k, v, W_line, v_param):
    """Host-side shard + layout prep. Returns per-core input maps."""
    qWq = q.astype(np.float64) @ W_line[:D].astype(np.float64)  # (B, H)
    wk = np.ascontiguousarray(W_line[D:])  # (D, H)
    vp4 = np.tile(v_param[:, None], (1, 4))  # (H, 4)

    in_maps = []
    for c in range(NCORES):
        bs = slice(c * BPC, (c + 1) * BPC)
        kT = np.ascontiguousarray(k[bs].transpose(0, 2, 1))  # (BPC, D, L)
        vv = np.zeros((BPC, L, DV), dtype=np.float32)
        vv[:, :, :D] = v[bs]
        vv[:, :, D] = 1.0
        qwq = np.ascontiguousarray(qWq[bs].T.astype(np.float32))  # (H, BPC)
        in_maps.append(
            {
                "kT": kT,
                "vv": vv,
                "wk": wk.astype(np.float32),
                "vp": vp4.astype(np.float32),
                "qwq": qwq,
            }
        )
    return in_maps


def _gather_output(results):
    out = np.empty((B, D), dtype=np.float32)
    for c, r in enumerate(results):
        rows = r["out"].reshape(BPC, DV).astype(np.float64)
        out[c * BPC : (c + 1) * BPC] = (rows[:, :D] / rows[:, D : D + 1]).astype(
            np.float32
        )
    return out


def run(q, k, v, W_line, v_param, trace=False, **spmd_kwargs):
    from concourse.bass_utils import run_bass_kernel_spmd

    if "nc" not in _CACHE:
        _CACHE["nc"] = build_nc()
    nc = _CACHE["nc"]
    in_maps = _prep_inputs(q, k, v, W_line, v_param)
    res = run_bass_kernel_spmd(
        nc, in_maps, list(range(NCORES)), trace=trace, **spmd_kwargs
    )
    return _gather_output(res.results), res


def kernel(q, k, v, W_line, v_param):
    out, _ = run(q, k, v, W_line, v_param, trace=False)
    return out


# revision 7
# speedup vs baseline: 1.2686x; 1.0044x over previous
"""Bahdanau-style additive attention on 8 TRN2 NeuronCores.

  hidden = tanh(q @ Wq + k @ Wk)        (B, L, H)
  scores = hidden @ v_param             (B, L)
  attn   = softmax(scores, axis=-1)
  out    = attn @ v                     (B, D)

Sharding: data-parallel over batch — 4 batches per core (B=32, 8 cores).

Per-core device pipeline (all heavy matmuls in float32r, the PE's
TF32-like 4-byte mode: ~11-bit-mantissa RNE inputs, fp32 accumulate):

  W1  preT[H, L]   = Wk.T @ kT          stationary=Wk, moving=host-transposed k
  ACT hiddenT      = tanh(preT + qWq_b) per-partition bias, f32r output
  W2  scores[L, 1] = hiddenT.T @ vp     stationary=hidden chunk -> score COLUMNS
  ACT w = exp(scores)                   no max-subtraction (|scores| << 88)
  W3  acc[1, D+1]  = w.T @ [v | 1]      stationary=w column, ones column gives
                                        the softmax denominator for free
  host: out = acc[:D] / acc[D]
"""

import numpy as np

import concourse.bass as bass
import concourse.mybir as mybir
from concourse.tile import TileContext

B, L, D, H = 32, 8192, 128, 128
NCORES = 8
BPC = B // NCORES  # batches per core
CHUNK = 512  # L positions per W1/tanh chunk (f32r moving max)
NCH = L // CHUNK  # 16 chunks per batch
KTILE = 2048  # L positions per kT DMA tile (1 MB transfers)
KCH = KTILE // CHUNK  # W1 chunks per kT tile
SUB = 128  # L positions per W2/W3 sub-chunk (stationary width)
NSUB = CHUNK // SUB  # 4
DV = 144  # v row: 128 data + ones col + pad to 576B (64B-aligned rows)
VT_COLS = 16  # W3 sub-chunks per v SBUF tile
NVT = L // (SUB * VT_COLS)  # 4 v tiles per batch

F32 = mybir.dt.float32
F32R = mybir.dt.float32r
ACTF = mybir.ActivationFunctionType

_CACHE = {}


def _split_excess_waits(nc, max_waits=1):
    """walrus in this env accepts at most one sync-wait per instruction;
    move extras onto InstNoOps placed just before (same engine, in order)."""
    for fn in nc.m.functions:
        for bb in fn.blocks:
            insts = list(bb.instructions)
            new_insts = []
            for ins in insts:
                si = ins.sync_info
                waits = list(si.on_wait) if si and si.on_wait else []
                if len(waits) > max_waits:
                    extra, keep = waits[:-max_waits], waits[-max_waits:]
                    for g0 in range(0, len(extra), max_waits):
                        pre = mybir.InstNoOp(
                            name=f"{ins.name}-waitsplit{g0}",
                            engine=ins.engine,
                            ins=[],
                            outs=[],
                            sync_info=mybir.SyncInfo(
                                on_wait=extra[g0 : g0 + max_waits], on_update=[]
                            ),
                        )
                        nc.register_instruction(pre, overwrite=True)
                        new_insts.append(pre)
                    ins.sync_info = mybir.SyncInfo(
                        on_wait=keep, on_update=list(si.on_update or [])
                    )
                new_insts.append(ins)
            if len(new_insts) != len(insts):
                bb.instructions[:] = new_insts


def build_nc():
    nc = bass.Bass("TRN2")

    kT_in = nc.dram_tensor("kT", [BPC, D, L], F32R, kind="ExternalInput")
    v_in = nc.dram_tensor("vv", [BPC, L, DV], F32R, kind="ExternalInput")
    wk_in = nc.dram_tensor("wk", [D, H], F32R, kind="ExternalInput")
    vp_in = nc.dram_tensor("vp", [H, 4], F32R, kind="ExternalInput")
    qwq_in = nc.dram_tensor("qwq", [H, BPC], F32, kind="ExternalInput")
    out_d = nc.dram_tensor("out", [1, BPC * DV], F32, kind="ExternalOutput")

    with TileContext(nc) as tc:
        with (
            tc.tile_pool(name="const", bufs=1) as cpool,
            tc.tile_pool(name="kp", bufs=4) as kpool,
            tc.tile_pool(name="vp_", bufs=2 * NVT) as vpool,
            tc.tile_pool(name="hp", bufs=3) as hpool,
            tc.tile_pool(name="sc", bufs=2) as scpool,
            tc.tile_pool(name="wp", bufs=2) as wpool,
            tc.tile_pool(name="ob", bufs=1) as opool,
            tc.tile_pool(name="pre", bufs=2, space="PSUM") as pre_pool,
            tc.tile_pool(name="sps", bufs=2, space="PSUM") as s_pool,
            tc.tile_pool(name="ops", bufs=2, space="PSUM") as o_pool,
        ):
            wk = cpool.tile([D, H], F32R)
            vp4 = cpool.tile([H, 4], F32R)
            qwq = cpool.tile([H, BPC], F32)
            nc.sync.dma_start(wk[:], wk_in[:])
            nc.sync.dma_start(vp4[:], vp_in[:])
            nc.sync.dma_start(qwq[:], qwq_in[:])

            out_sb = opool.tile([1, BPC * DV], F32)

            # HAM warm-up: ~8us of back-to-back PE work while the first kT
            # tile is still in flight; lifts the PE clock gate to 8/8.
            warm_ps = o_pool.tile([H, 4], F32, tag="warm")
            for _ in range(40):
                nc.tensor.matmul(warm_ps[:], wk[:], vp4[:], start=True, stop=True)

            for b in range(BPC):
                # v~ tiles for this batch (SWDGE queue so the large v
                # prefetches never head-of-line-block the kT stream, whose
                # issue rate is throttled by PE back-pressure)
                v_tiles = [None] * NVT

                scores = scpool.tile([SUB, L // SUB], F32, tag="scores")
                kts = {}
                for t in range(NCH):
                    if t % KCH == 0:
                        ktile = kpool.tile([D, KTILE], F32R, tag="kt")
                        nc.sync.dma_start(
                            ktile[:], kT_in[b, :, t * CHUNK : t * CHUNK + KTILE]
                        )
                        kts[t // KCH] = ktile
                    if t % 4 == 1:
                        vt = t // 4
                        vtile = vpool.tile([SUB, VT_COLS * DV], F32R, tag="vt")
                        src = v_in[
                            b, vt * SUB * VT_COLS : (vt + 1) * SUB * VT_COLS, :
                        ]
                        nc.gpsimd.dma_start(
                            vtile[:].rearrange("p (t d) -> p t d", d=DV),
                            src.rearrange("(t p) d -> p t d", p=SUB),
                        )
                        v_tiles[vt] = vtile
                    kt = kts[t // KCH][:, (t % KCH) * CHUNK : (t % KCH + 1) * CHUNK]
                    pre = pre_pool.tile([H, CHUNK], F32, tag="pre")
                    nc.tensor.matmul(pre[:], wk[:], kt, start=True, stop=True)
                    hid = hpool.tile([H, CHUNK], F32R, tag="hid")
                    nc.scalar.activation(
                        hid[:], pre[:], ACTF.Tanh, bias=qwq[:, b : b + 1], scale=1.0
                    )
                    scol = s_pool.tile([SUB, 4 * NSUB], F32, tag="scol")
                    for j in range(NSUB):
                        nc.tensor.matmul(
                            scol[:, 4 * j : 4 * j + 4],
                            hid[:, j * SUB : (j + 1) * SUB],
                            vp4[:],
                            start=True,
                            stop=True,
                        )
                    # gather the 4 useful columns {0,4,8,12} -> scores[:, 4t..4t+4)
                    nc.vector.tensor_copy(
                        scores[:, 4 * t : 4 * t + 4], scol[:, 0 : 4 * NSUB : 4]
                    )

                w = wpool.tile([SUB, L // SUB], F32R, tag="w")
                nc.scalar.activation(w[:], scores[:], ACTF.Exp)

                acc = o_pool.tile([1, DV], F32, tag="acc")
                nsub_total = L // SUB
                for tp in range(nsub_total):
                    vt, col = divmod(tp, VT_COLS)
                    nc.tensor.matmul(
                        acc[:],
                        w[:, tp : tp + 1],
                        v_tiles[vt][:, col * DV : (col + 1) * DV],
                        start=(tp == 0),
                        stop=(tp == nsub_total - 1),
                    )
                nc.scalar.copy(out_sb[:, b * DV : (b + 1) * DV], acc[:])

            nc.sync.dma_start(out_d[:], out_sb[:])

    _split_excess_waits(nc)
    return nc


def _prep_inputs(q, k, v, W_line, v_param):
    """Host-side shard + layout prep. Returns per-core input maps."""
    qWq = q.astype(np.float64) @ W_line[:D].astype(np.float64)  # (B, H)
    wk = np.ascontiguousarray(W_line[D:])  # (D, H)
    vp4 = np.tile(v_param[:, None], (1, 4))  # (H, 4)

    in_maps = []
    for c in range(NCORES):
        bs = slice(c * BPC, (c + 1) * BPC)
        kT = np.ascontiguousarray(k[bs].transpose(0, 2, 1))  # (BPC, D, L)
        vv = np.zeros((BPC, L, DV), dtype=np.float32)
        vv[:, :, :D] = v[bs]
        vv[:, :, D] = 1.0
        qwq = np.ascontiguousarray(qWq[bs].T.astype(np.float32))  # (H, BPC)
        in_maps.append(
            {
                "kT": kT,
                "vv": vv,
                "wk": wk.astype(np.float32),
                "vp": vp4.astype(np.float32),
                "qwq": qwq,
            }
        )
    return in_maps


def _gather_output(results):
    out = np.empty((B, D), dtype=np.float32)
    for c, r in enumerate(results):
        rows = r["out"].reshape(BPC, DV).astype(np.float64)
        out[c * BPC : (c + 1) * BPC] = (rows[:, :D] / rows[:, D : D + 1]).astype(
            np.float32
        )
    return out


def run(q, k, v, W_line, v_param, trace=False, **spmd_kwargs):
    from concourse.bass_utils import run_bass_kernel_spmd

    if "nc" not in _CACHE:
        _CACHE["nc"] = build_nc()
    nc = _CACHE["nc"]
    in_maps = _prep_inputs(q, k, v, W_line, v_param)
    res = run_bass_kernel_spmd(
        nc, in_maps, list(range(NCORES)), trace=trace, **spmd_kwargs
    )
    return _gather_output(res.results), res


def kernel(q, k, v, W_line, v_param):
    out, _ = run(q, k, v, W_line, v_param, trace=False)
    return out
